# revision 23
# baseline (speedup 1.0000x reference)
"""Trainium2 Bass kernel for a fused LSTM cell.

Reference math (B=8192, D=U=1024, all fp32):
    z = x @ Wx + h_tm1 @ Uh + b          # Wx=[W_i|W_f|W_c|W_o], Uh likewise
    i, f = sigmoid(z_i), sigmoid(z_f)
    c = f * c_tm1 + i * tanh(z_c)
    h = sigmoid(z_o) * tanh(c)
    returns (h, c)

Strategy:
  - Data-parallel over 8 NeuronCores: batch 8192 -> 1024 rows/core,
    weights replicated. No collectives.
  - Per core the GEMM is computed transposed: z^T [4096 units, 1024 batch].
    lhsT (stationary) = weight tiles [128k, 128n] in natural [K, N] layout;
    rhs (moving) = host-pretransposed [x|h]^T tiles [128k, 512 batch].
    This puts units on PSUM partitions so the per-unit bias becomes a
    per-partition scalar folded into the ScalarE activation for free.
  - Matmul operands are fp16 (PSUM accumulation fp32): same 1 cycle/row
    PE rate as fp32r, but halves HBM traffic and enables the compiler's
    fast-weight-load path so LDWEIGHTS hides under the matmul stream.
    Worst-case |err| vs the fp32 reference ~2.3e-3 of max|h|.
  - Per (j, bh) block the i/f/c gate chains run first; the c-state
    combine, tanh(c) and the c DMA overlap the o-gate's matmul chain, so
    only act(z_o) -> h mul -> h DMA trail the final matmul.
"""

from contextlib import ExitStack

import numpy as np

import concourse.bass as bass
import concourse.tile as tile
from concourse import bacc, mybir
from concourse.bass_utils import run_bass_kernel_spmd

B, D, U = 8192, 1024, 1024
NCORES = 8
BS = B // NCORES  # per-core batch rows


def build_nc(bs=BS, d=D, u=U, f=512):
    """Build the per-core SPMD Bass program.

    DRAM parameter layouts (host prepares these):
      xh   [KO, BH, 128, f] fp16 : [x|h]^T, contraction dim on (KO, partition)
      w    [JB, KO, 128, 4, 128] fp16 : w[j, ko, p, g, n] = W_all[ko*128+p, (g*JB+j)*128+n]
      bias [128, NT] fp32        : bias[p, t] = b_all[t*128+p]
      ct   [JB, 128, bs] fp32    : c_tm1^T unit-blocks
      h_out/c_out [JB, 128, bs] fp32 : h^T / c^T unit-blocks
    """
    kdim = d + u
    KO = kdim // 128   # contraction blocks
    KO2 = KO // 2      # paired contraction blocks (2KB DMA rows)
    JB = u // 128      # unit blocks per gate
    NT = 4 * u // 128  # total n-tiles (4 gates)
    f = min(f, bs)
    BH = bs // f       # batch chunks of the moving operand

    f32 = mybir.dt.float32
    f16 = mybir.dt.float16
    SIG = mybir.ActivationFunctionType.Sigmoid
    TANH = mybir.ActivationFunctionType.Tanh

    nc = bacc.Bacc("TRN2", target_bir_lowering=False, debug=False)

    # 2KB per-partition DMA rows (fp16): whole-ko xh rows, ko-paired w rows
    xh = nc.dram_tensor("xh", [KO, 128, bs], f16, kind="ExternalInput").ap()
    # w[j, ko2, p, c, g, n] = W_all[(2*ko2+c)*128+p, (g*JB+j)*128+n]
    w = nc.dram_tensor("w", [JB, KO2, 128, 2, 4, 128], f16, kind="ExternalInput").ap()
    bia = nc.dram_tensor("bias", [128, NT], f32, kind="ExternalInput").ap()
    ct = nc.dram_tensor("ct", [JB, 128, bs], f32, kind="ExternalInput").ap()
    ho = nc.dram_tensor("h_out", [JB, 128, bs], f32, kind="ExternalOutput").ap()
    co = nc.dram_tensor("c_out", [JB, 128, bs], f32, kind="ExternalOutput").ap()

    with tile.TileContext(nc) as tc, ExitStack() as ctx:
        xh_pool = ctx.enter_context(tc.tile_pool(name="xh", bufs=1))
        w_pool = ctx.enter_context(tc.tile_pool(name="w", bufs=2 * KO2))
        bias_pool = ctx.enter_context(tc.tile_pool(name="bias", bufs=1))
        ct_pool = ctx.enter_context(tc.tile_pool(name="ct", bufs=2))
        gate_pool = ctx.enter_context(tc.tile_pool(name="gates", bufs=2))
        out_pool = ctx.enter_context(tc.tile_pool(name="outs", bufs=2))
        psum_pool = ctx.enter_context(tc.tile_pool(name="psum", bufs=8, space="PSUM"))

        # Warm-up: the PE idles ~8us waiting for the first DMAs, long
        # enough for the HAM activity monitor to hold it at 1.2 GHz.
        # Junk matmuls on a memset tile (into j=0's first PSUM bank,
        # reset by the real chain's start=True) span that window so the
        # real stream opens at the warm 2.4 GHz clock.
        warm = bias_pool.tile([128, f], f16, tag="warm")
        nc.vector.memset(warm[:], 0)
        ps0 = [
            [
                psum_pool.tile([128, f], f32, tag="ps", name=f"ps_{g}_{bh}")
                for bh in range(BH)
            ]
            for g in range(4)
        ]
        for _ in range(16):
            nc.tensor.matmul(
                ps0[0][0][:], lhsT=warm[:, :128], rhs=warm[:], start=True, stop=True
            )

        bias_sb = bias_pool.tile([128, NT], f32, tag="bias")

        def load_ct(j):
            t = ct_pool.tile([128, bs], f32, tag="ct")
            nc.sync.dma_start(t[:], ct[j])
            return t

        def load_wk(j, ko2):
            t = w_pool.tile([128, 2, 4, 128], f16, tag="wk", name=f"wk_{j}_{ko2}")
            nc.sync.dma_start(t[:], w[j, ko2])
            return t

        def wslice(wk, ko, g):
            return wk[ko // 2][:, ko % 2, g, :]

        # Startup: j=0's weights interleave with xh so PE starts after
        # the first chunks instead of after the full stream prefix.
        xh_sb = []
        wk_by_j = {0: []}
        for ko2 in range(KO2):
            wk_by_j[0].append(load_wk(0, ko2))
            for ko in (2 * ko2, 2 * ko2 + 1):
                t = xh_pool.tile([128, bs], f16, tag=f"xh{ko}", name=f"xh{ko}")
                nc.sync.dma_start(t[:], xh[ko])
                xh_sb.append(t)
            if ko2 == 0:
                # bias isn't needed until j=0's epilogue; keep its DMA
                # trigger behind the first matmul's operands
                nc.sync.dma_start(bias_sb[:], bia[:])
        ct_by_j = {0: load_ct(0)}  # not needed until j=0's epilogue

        def act_gate(j, g, ps, width=None):
            n = f if width is None else width
            gtile = gate_pool.tile([128, n], f32, tag=f"g{g}_{n}")
            idx = g * JB + j
            func = TANH if g == 2 else SIG
            nc.scalar.activation(
                gtile[:], ps[:, :n], func, bias=bias_sb[:, idx : idx + 1]
            )
            return gtile

        def combine_c(j, bh, gi, gf, gc, ct_sb, c_out):
            """c = f*c_tm1 + i*tanh(zc); DMA c out; return tanh(c).

            Issued before the o-gate chain's activation so ScalarE's FIFO
            doesn't stall tanh(c) behind act(z_o).
            """
            bsl = slice(bh * f, (bh + 1) * f)
            t1 = gate_pool.tile([128, f], f32, tag="t1")
            nc.vector.tensor_mul(t1[:], gf[:], ct_sb[:, bsl])
            t2 = gate_pool.tile([128, f], f32, tag="t2")
            nc.vector.tensor_mul(t2[:], gi[:], gc[:])
            nc.vector.tensor_add(c_out[:, bsl], t1[:], t2[:])
            nc.sync.dma_start(co[j][:, bsl], c_out[:, bsl])
            tct = gate_pool.tile([128, f], f32, tag="tct")
            nc.scalar.activation(tct[:], c_out[:, bsl], TANH)
            return tct

        def finish_h(j, bh, go, tct, h_out, cols=None):
            csl = slice(0, f) if cols is None else cols
            bsl = slice(bh * f + csl.start, bh * f + csl.stop)
            nc.vector.tensor_mul(h_out[:, bsl], go[:], tct[:, csl])
            nc.sync.dma_start(ho[j][:, bsl], h_out[:, bsl])

        for j in range(JB):
            # prefetch next block's weights/ct one block ahead
            if j + 1 < JB and (j + 1) not in wk_by_j:
                wk_by_j[j + 1] = [load_wk(j + 1, ko2) for ko2 in range(KO2)]
            if j + 1 < JB and (j + 1) not in ct_by_j:
                ct_by_j[j + 1] = load_ct(j + 1)
            wk = wk_by_j.pop(j)
            ct_sb = ct_by_j.pop(j)
            h_out = out_pool.tile([128, bs], f32, tag="h")
            c_out = out_pool.tile([128, bs], f32, tag="c")
            if j == 0:
                # ko-major: all 8 (g, bh) groups accumulate together so the
                # PE chases the arriving xh/w DMAs instead of waiting for
                # the whole prefix.
                ps = ps0
                for ko in range(KO):
                    for bh in range(BH):
                        bsl = slice(bh * f, (bh + 1) * f)
                        for g in range(4):
                            nc.tensor.matmul(
                                ps[g][bh][:],
                                lhsT=wslice(wk, ko, g),
                                rhs=xh_sb[ko][:, bsl],
                                start=(ko == 0),
                                stop=(ko == KO - 1),
                            )
                for bh in range(BH):
                    gi = act_gate(j, 0, ps[0][bh])
                    gf = act_gate(j, 1, ps[1][bh])
                    gc = act_gate(j, 2, ps[2][bh])
                    tct = combine_c(j, bh, gi, gf, gc, ct_sb, c_out)
                    go = act_gate(j, 3, ps[3][bh])
                    finish_h(j, bh, go, tct, h_out)
            else:
                for bh in range(BH):
                    bsl_f = slice(bh * f, (bh + 1) * f)

                    def chain(g):
                        psb = psum_pool.tile([128, f], f32, tag="ps")
                        for ko in range(KO):
                            nc.tensor.matmul(
                                psb[:],
                                lhsT=wslice(wk, ko, g),
                                rhs=xh_sb[ko][:, bsl_f],
                                start=(ko == 0),
                                stop=(ko == KO - 1),
                            )
                        return psb

                    gi = act_gate(j, 0, chain(0))
                    gf = act_gate(j, 1, chain(1))
                    gc = act_gate(j, 2, chain(2))
                    # c-state combine + tanh + c DMA overlap the o chain
                    tct = combine_c(j, bh, gi, gf, gc, ct_sb, c_out)
                    if j == JB - 1 and bh == BH - 1:
                        # split the last o chain into narrow accumulations:
                        # earlier chunks' act+mul+DMA hide under later
                        # chunks' matmuls, so only a 128-wide epilogue
                        # trails the final matmul
                        NCH = 4
                        for ci in range(NCH):
                            csl = slice(ci * (f // NCH), (ci + 1) * (f // NCH))
                            bcsl = slice(bh * f + csl.start, bh * f + csl.stop)
                            psb = psum_pool.tile(
                                [128, f // NCH], f32, tag="ps", name=f"pso{ci}"
                            )
                            for ko in range(KO):
                                nc.tensor.matmul(
                                    psb[:],
                                    lhsT=wslice(wk, ko, 3),
                                    rhs=xh_sb[ko][:, bcsl],
                                    start=(ko == 0),
                                    stop=(ko == KO - 1),
                                )
                            go = act_gate(j, 3, psb, width=f // NCH)
                            finish_h(j, bh, go, tct, h_out, cols=csl)
                    else:
                        go = act_gate(j, 3, chain(3))
                        finish_h(j, bh, go, tct, h_out)

    nc.compile()
    return nc


def pack_shared(inputs):
    """Weight + bias device arrays (replicated on every core)."""
    d, u = inputs["W_i"].shape[0], inputs["W_i"].shape[1]
    kdim = d + u
    KO = kdim // 128
    NT = 4 * u // 128
    Wx = np.concatenate(
        [inputs["W_i"], inputs["W_f"], inputs["W_c"], inputs["W_o"]], axis=1
    )
    Uh = np.concatenate(
        [inputs["U_i"], inputs["U_f"], inputs["U_c"], inputs["U_o"]], axis=1
    )
    W_all = np.concatenate([Wx, Uh], axis=0)  # [kdim, 4u]
    JB = u // 128
    # w_dev[j, ko2, p, c, g, n] = W_all[(2*ko2+c)*128+p, (g*JB+j)*128+n]
    w_dev = np.ascontiguousarray(
        W_all.reshape(KO // 2, 2, 128, 4, JB, 128).transpose(4, 0, 2, 1, 3, 5)
    ).astype(np.float16)
    b_all = np.concatenate(
        [inputs["b_i"], inputs["b_f"], inputs["b_c"], inputs["b_o"]]
    )  # [4u]
    b_dev = np.ascontiguousarray(b_all.reshape(NT, 128).T).astype(np.float32)
    return w_dev, b_dev


def pack_core(x_i, h_i, c_i, f=512):
    """Per-core shard arrays."""
    bs = x_i.shape[0]
    d, u = x_i.shape[1], h_i.shape[1]
    KO = (d + u) // 128
    JB = u // 128
    f = min(f, bs)
    BH = bs // f
    xh_t = np.concatenate([x_i, h_i], axis=1).T  # [kdim, bs]
    xh_dev = np.ascontiguousarray(xh_t.reshape(KO, 128, bs)).astype(np.float16)
    ct_dev = np.ascontiguousarray(c_i.T.reshape(JB, 128, bs)).astype(np.float32)
    return xh_dev, ct_dev


_NC_CACHE = {}


def _get_nc():
    key = (BS, D, U)
    if key not in _NC_CACHE:
        _NC_CACHE[key] = build_nc()
    return _NC_CACHE[key]


def build_in_maps(inputs, ncores=NCORES):
    x = np.asarray(inputs["inputs"], np.float32)
    h = np.asarray(inputs["h_tm1"], np.float32)
    c = np.asarray(inputs["c_tm1"], np.float32)
    w_dev, b_dev = pack_shared(inputs)
    in_maps = []
    for i in range(ncores):
        sl = slice(i * BS, (i + 1) * BS)
        xh_dev, ct_dev = pack_core(x[sl], h[sl], c[sl])
        in_maps.append({"xh": xh_dev, "w": w_dev, "bias": b_dev, "ct": ct_dev})
    return in_maps


def _run(inputs, trace=False):
    in_maps = build_in_maps(inputs)
    nc = _get_nc()
    res = run_bass_kernel_spmd(nc, in_maps, list(range(NCORES)), trace=trace)
    u = U
    h_full = np.empty((B, u), np.float32)
    c_full = np.empty((B, u), np.float32)
    for i in range(NCORES):
        sl = slice(i * BS, (i + 1) * BS)
        h_full[sl] = res.results[i]["h_out"].reshape(u, BS).T
        c_full[sl] = res.results[i]["c_out"].reshape(u, BS).T
    return (h_full, c_full), res


def kernel(**inputs):
    out, _ = _run(inputs, trace=False)
    return out


# revision 28
# speedup vs baseline: 1.0089x; 1.0089x over previous
"""Trainium2 Bass kernel for a fused LSTM cell.

Reference math (B=8192, D=U=1024, all fp32):
    z = x @ Wx + h_tm1 @ Uh + b          # Wx=[W_i|W_f|W_c|W_o], Uh likewise
    i, f = sigmoid(z_i), sigmoid(z_f)
    c = f * c_tm1 + i * tanh(z_c)
    h = sigmoid(z_o) * tanh(c)
    returns (h, c)

Strategy:
  - Data-parallel over 8 NeuronCores: batch 8192 -> 1024 rows/core,
    weights replicated. No collectives.
  - Per core the GEMM is computed transposed: z^T [4096 units, 1024 batch].
    lhsT (stationary) = weight tiles [128k, 128n] in natural [K, N] layout;
    rhs (moving) = host-pretransposed [x|h]^T tiles [128k, 512 batch].
    This puts units on PSUM partitions so the per-unit bias becomes a
    per-partition scalar folded into the ScalarE activation for free.
  - Matmul operands are fp16 (PSUM accumulation fp32): same 1 cycle/row
    PE rate as fp32r, but halves HBM traffic and enables the compiler's
    fast-weight-load path so LDWEIGHTS hides under the matmul stream.
    Worst-case |err| vs the fp32 reference ~2.3e-3 of max|h|.
  - Per (j, bh) block the i/f/c gate chains run first; the c-state
    combine, tanh(c) and the c DMA overlap the o-gate's matmul chain, so
    only act(z_o) -> h mul -> h DMA trail the final matmul.
"""

from contextlib import ExitStack

import numpy as np

import concourse.bass as bass
import concourse.tile as tile
from concourse import bacc, mybir
from concourse.bass_utils import run_bass_kernel_spmd

B, D, U = 8192, 1024, 1024
NCORES = 8
BS = B // NCORES  # per-core batch rows


def build_nc(bs=BS, d=D, u=U, f=512):
    """Build the per-core SPMD Bass program.

    DRAM parameter layouts (host prepares these):
      xh   [KO, BH, 128, f] fp16 : [x|h]^T, contraction dim on (KO, partition)
      w    [JB, KO, 128, 4, 128] fp16 : w[j, ko, p, g, n] = W_all[ko*128+p, (g*JB+j)*128+n]
      bias [128, NT] fp32        : bias[p, t] = b_all[t*128+p]
      ct   [JB, 128, bs] fp32    : c_tm1^T unit-blocks
      h_out/c_out [JB, 128, bs] fp32 : h^T / c^T unit-blocks
    """
    kdim = d + u
    KO = kdim // 128   # contraction blocks
    KO2 = KO // 2      # paired contraction blocks (2KB DMA rows)
    JB = u // 128      # unit blocks per gate
    NT = 4 * u // 128  # total n-tiles (4 gates)
    f = min(f, bs)
    BH = bs // f       # batch chunks of the moving operand

    f32 = mybir.dt.float32
    f16 = mybir.dt.float16
    SIG = mybir.ActivationFunctionType.Sigmoid
    TANH = mybir.ActivationFunctionType.Tanh

    nc = bacc.Bacc("TRN2", target_bir_lowering=False, debug=False)

    # 2KB per-partition DMA rows (fp16): whole-ko xh rows, ko-paired w rows
    xh = nc.dram_tensor("xh", [KO, 128, bs], f16, kind="ExternalInput").ap()
    # w[j, ko2, p, c, g, n] = W_all[(2*ko2+c)*128+p, (g*JB+j)*128+n]
    w = nc.dram_tensor("w", [JB, KO2, 128, 2, 4, 128], f16, kind="ExternalInput").ap()
    bia = nc.dram_tensor("bias", [128, NT], f32, kind="ExternalInput").ap()
    ct = nc.dram_tensor("ct", [JB, 128, bs], f32, kind="ExternalInput").ap()
    ho = nc.dram_tensor("h_out", [JB, 128, bs], f32, kind="ExternalOutput").ap()
    co = nc.dram_tensor("c_out", [JB, 128, bs], f32, kind="ExternalOutput").ap()

    with tile.TileContext(nc) as tc, ExitStack() as ctx:
        xh_pool = ctx.enter_context(tc.tile_pool(name="xh", bufs=1))
        w_pool = ctx.enter_context(tc.tile_pool(name="w", bufs=2 * KO2))
        bias_pool = ctx.enter_context(tc.tile_pool(name="bias", bufs=1))
        ct_pool = ctx.enter_context(tc.tile_pool(name="ct", bufs=2))
        gate_pool = ctx.enter_context(tc.tile_pool(name="gates", bufs=2))
        out_pool = ctx.enter_context(tc.tile_pool(name="outs", bufs=2))
        psum_pool = ctx.enter_context(tc.tile_pool(name="psum", bufs=8, space="PSUM"))

        # Warm-up: the PE idles ~8us waiting for the first DMAs, long
        # enough for the HAM activity monitor to hold it at 1.2 GHz.
        # Junk matmuls on a memset tile (into j=0's first PSUM bank,
        # reset by the real chain's start=True) span that window so the
        # real stream opens at the warm 2.4 GHz clock.
        warm = bias_pool.tile([128, f], f16, tag="warm")
        nc.vector.memset(warm[:], 0)
        ps0 = [
            [
                psum_pool.tile([128, f], f32, tag="ps", name=f"ps_{g}_{bh}")
                for bh in range(BH)
            ]
            for g in range(4)
        ]
        for _ in range(14):
            nc.tensor.matmul(
                ps0[0][0][:], lhsT=warm[:, :128], rhs=warm[:], start=True, stop=True
            )

        bias_sb = bias_pool.tile([128, NT], f32, tag="bias")

        def load_ct(j):
            t = ct_pool.tile([128, bs], f32, tag="ct")
            nc.sync.dma_start(t[:], ct[j])
            return t

        def load_wk(j, ko2):
            t = w_pool.tile([128, 2, 4, 128], f16, tag="wk", name=f"wk_{j}_{ko2}")
            nc.sync.dma_start(t[:], w[j, ko2])
            return t

        def wslice(wk, ko, g):
            return wk[ko // 2][:, ko % 2, g, :]

        # Startup: j=0's weights interleave with xh so PE starts after
        # the first chunks instead of after the full stream prefix.
        xh_sb = []
        wk_by_j = {0: []}
        for ko2 in range(KO2):
            if ko2 == 0:
                # halve the very first loads so the first matmuls (which
                # read only the ko=0 / bh=0 sub-regions) start ~1us sooner
                t = w_pool.tile([128, 2, 4, 128], f16, tag="wk", name="wk_0_0")
                nc.sync.dma_start(t[:, 0], w[0, 0][:, 0])
                wk_by_j[0].append(t)
                tx = xh_pool.tile([128, bs], f16, tag="xh0", name="xh0")
                nc.sync.dma_start(tx[:, : bs // 2], xh[0][:, : bs // 2])
                nc.sync.dma_start(t[:, 1], w[0, 0][:, 1])
                nc.sync.dma_start(tx[:, bs // 2 :], xh[0][:, bs // 2 :])
                xh_sb.append(tx)
                tx = xh_pool.tile([128, bs], f16, tag="xh1", name="xh1")
                nc.sync.dma_start(tx[:], xh[1])
                xh_sb.append(tx)
                # bias isn't needed until j=0's epilogue; keep its DMA
                # trigger behind the first matmul's operands
                nc.sync.dma_start(bias_sb[:], bia[:])
                continue
            wk_by_j[0].append(load_wk(0, ko2))
            for ko in (2 * ko2, 2 * ko2 + 1):
                t = xh_pool.tile([128, bs], f16, tag=f"xh{ko}", name=f"xh{ko}")
                nc.sync.dma_start(t[:], xh[ko])
                xh_sb.append(t)
        ct_by_j = {0: load_ct(0)}  # not needed until j=0's epilogue

        def act_gate(j, g, ps, width=None):
            n = f if width is None else width
            gtile = gate_pool.tile([128, n], f32, tag=f"g{g}_{n}")
            idx = g * JB + j
            func = TANH if g == 2 else SIG
            nc.scalar.activation(
                gtile[:], ps[:, :n], func, bias=bias_sb[:, idx : idx + 1]
            )
            return gtile

        def combine_c(j, bh, gi, gf, gc, ct_sb, c_out):
            """c = f*c_tm1 + i*tanh(zc); DMA c out; return tanh(c).

            Issued before the o-gate chain's activation so ScalarE's FIFO
            doesn't stall tanh(c) behind act(z_o).
            """
            bsl = slice(bh * f, (bh + 1) * f)
            t1 = gate_pool.tile([128, f], f32, tag="t1")
            nc.vector.tensor_mul(t1[:], gf[:], ct_sb[:, bsl])
            t2 = gate_pool.tile([128, f], f32, tag="t2")
            nc.vector.tensor_mul(t2[:], gi[:], gc[:])
            nc.vector.tensor_add(c_out[:, bsl], t1[:], t2[:])
            nc.sync.dma_start(co[j][:, bsl], c_out[:, bsl])
            tct = gate_pool.tile([128, f], f32, tag="tct")
            nc.scalar.activation(tct[:], c_out[:, bsl], TANH)
            return tct

        def finish_h(j, bh, go, tct, h_out, cols=None, dma_cols=None):
            csl = slice(0, f) if cols is None else cols
            bsl = slice(bh * f + csl.start, bh * f + csl.stop)
            nc.vector.tensor_mul(h_out[:, bsl], go[:], tct[:, csl])
            if dma_cols is False:
                return
            dsl = csl if dma_cols is None else dma_cols
            bdsl = slice(bh * f + dsl.start, bh * f + dsl.stop)
            nc.sync.dma_start(ho[j][:, bdsl], h_out[:, bdsl])

        for j in range(JB):
            # prefetch next block's weights/ct one block ahead
            if j + 1 < JB and (j + 1) not in wk_by_j:
                wk_by_j[j + 1] = [load_wk(j + 1, ko2) for ko2 in range(KO2)]
            if j + 1 < JB and (j + 1) not in ct_by_j:
                ct_by_j[j + 1] = load_ct(j + 1)
            wk = wk_by_j.pop(j)
            ct_sb = ct_by_j.pop(j)
            h_out = out_pool.tile([128, bs], f32, tag="h")
            c_out = out_pool.tile([128, bs], f32, tag="c")
            if j == 0:
                # ko-major: all 8 (g, bh) groups accumulate together so the
                # PE chases the arriving xh/w DMAs instead of waiting for
                # the whole prefix.
                ps = ps0
                for ko in range(KO):
                    for bh in range(BH):
                        bsl = slice(bh * f, (bh + 1) * f)
                        for g in range(4):
                            nc.tensor.matmul(
                                ps[g][bh][:],
                                lhsT=wslice(wk, ko, g),
                                rhs=xh_sb[ko][:, bsl],
                                start=(ko == 0),
                                stop=(ko == KO - 1),
                            )
                for bh in range(BH):
                    gi = act_gate(j, 0, ps[0][bh])
                    gf = act_gate(j, 1, ps[1][bh])
                    gc = act_gate(j, 2, ps[2][bh])
                    tct = combine_c(j, bh, gi, gf, gc, ct_sb, c_out)
                    go = act_gate(j, 3, ps[3][bh])
                    finish_h(j, bh, go, tct, h_out)
            else:
                for bh in range(BH):
                    bsl_f = slice(bh * f, (bh + 1) * f)

                    def chain(g):
                        psb = psum_pool.tile([128, f], f32, tag="ps")
                        for ko in range(KO):
                            nc.tensor.matmul(
                                psb[:],
                                lhsT=wslice(wk, ko, g),
                                rhs=xh_sb[ko][:, bsl_f],
                                start=(ko == 0),
                                stop=(ko == KO - 1),
                            )
                        return psb

                    gi = act_gate(j, 0, chain(0))
                    gf = act_gate(j, 1, chain(1))
                    gc = act_gate(j, 2, chain(2))
                    # c-state combine + tanh + c DMA overlap the o chain
                    tct = combine_c(j, bh, gi, gf, gc, ct_sb, c_out)
                    if j == JB - 1 and bh == BH - 1:
                        # split the last o chain into narrow accumulations:
                        # earlier chunks' act+mul+DMA hide under later
                        # chunks' matmuls, so only a 128-wide epilogue
                        # trails the final matmul
                        NCH = 4
                        for ci in range(NCH):
                            csl = slice(ci * (f // NCH), (ci + 1) * (f // NCH))
                            bcsl = slice(bh * f + csl.start, bh * f + csl.stop)
                            psb = psum_pool.tile(
                                [128, f // NCH], f32, tag="ps", name=f"pso{ci}"
                            )
                            for ko in range(KO):
                                nc.tensor.matmul(
                                    psb[:],
                                    lhsT=wslice(wk, ko, 3),
                                    rhs=xh_sb[ko][:, bcsl],
                                    start=(ko == 0),
                                    stop=(ko == KO - 1),
                                )
                            go = act_gate(j, 3, psb, width=f // NCH)
                            # one h DMA per chunk pair: fewer serialized
                            # ~600ns triggers on the Sync queue at the tail
                            dma_cols = (
                                slice((ci - 1) * (f // NCH), (ci + 1) * (f // NCH))
                                if ci % 2 == 1
                                else False
                            )
                            finish_h(
                                j, bh, go, tct, h_out, cols=csl,
                                dma_cols=dma_cols,
                            )
                    else:
                        go = act_gate(j, 3, chain(3))
                        finish_h(j, bh, go, tct, h_out)

    nc.compile()
    return nc


def pack_shared(inputs):
    """Weight + bias device arrays (replicated on every core)."""
    d, u = inputs["W_i"].shape[0], inputs["W_i"].shape[1]
    kdim = d + u
    KO = kdim // 128
    NT = 4 * u // 128
    Wx = np.concatenate(
        [inputs["W_i"], inputs["W_f"], inputs["W_c"], inputs["W_o"]], axis=1
    )
    Uh = np.concatenate(
        [inputs["U_i"], inputs["U_f"], inputs["U_c"], inputs["U_o"]], axis=1
    )
    W_all = np.concatenate([Wx, Uh], axis=0)  # [kdim, 4u]
    JB = u // 128
    # w_dev[j, ko2, p, c, g, n] = W_all[(2*ko2+c)*128+p, (g*JB+j)*128+n]
    w_dev = np.ascontiguousarray(
        W_all.reshape(KO // 2, 2, 128, 4, JB, 128).transpose(4, 0, 2, 1, 3, 5)
    ).astype(np.float16)
    b_all = np.concatenate(
        [inputs["b_i"], inputs["b_f"], inputs["b_c"], inputs["b_o"]]
    )  # [4u]
    b_dev = np.ascontiguousarray(b_all.reshape(NT, 128).T).astype(np.float32)
    return w_dev, b_dev


def pack_core(x_i, h_i, c_i, f=512):
    """Per-core shard arrays."""
    bs = x_i.shape[0]
    d, u = x_i.shape[1], h_i.shape[1]
    KO = (d + u) // 128
    JB = u // 128
    f = min(f, bs)
    BH = bs // f
    xh_t = np.concatenate([x_i, h_i], axis=1).T  # [kdim, bs]
    xh_dev = np.ascontiguousarray(xh_t.reshape(KO, 128, bs)).astype(np.float16)
    ct_dev = np.ascontiguousarray(c_i.T.reshape(JB, 128, bs)).astype(np.float32)
    return xh_dev, ct_dev


_NC_CACHE = {}


def _get_nc():
    key = (BS, D, U)
    if key not in _NC_CACHE:
        _NC_CACHE[key] = build_nc()
    return _NC_CACHE[key]


def build_in_maps(inputs, ncores=NCORES):
    x = np.asarray(inputs["inputs"], np.float32)
    h = np.asarray(inputs["h_tm1"], np.float32)
    c = np.asarray(inputs["c_tm1"], np.float32)
    w_dev, b_dev = pack_shared(inputs)
    in_maps = []
    for i in range(ncores):
        sl = slice(i * BS, (i + 1) * BS)
        xh_dev, ct_dev = pack_core(x[sl], h[sl], c[sl])
        in_maps.append({"xh": xh_dev, "w": w_dev, "bias": b_dev, "ct": ct_dev})
    return in_maps


def _run(inputs, trace=False):
    in_maps = build_in_maps(inputs)
    nc = _get_nc()
    res = run_bass_kernel_spmd(nc, in_maps, list(range(NCORES)), trace=trace)
    u = U
    h_full = np.empty((B, u), np.float32)
    c_full = np.empty((B, u), np.float32)
    for i in range(NCORES):
        sl = slice(i * BS, (i + 1) * BS)
        h_full[sl] = res.results[i]["h_out"].reshape(u, BS).T
        c_full[sl] = res.results[i]["c_out"].reshape(u, BS).T
    return (h_full, c_full), res


def kernel(**inputs):
    out, _ = _run(inputs, trace=False)
    return out
